# revision 2
# baseline (speedup 1.0000x reference)
"""HGT on 8 trn2 NeuronCores — bf16, two-pass phase-2 pipeline (v5).

Dst-sharded node partition; kv tables split by src node half (kvA/kvB,
per-layer tensors). Phase 2 runs in two passes over all windows:
  pass 1: low-half columns only; partial (agg|ez-sum) spilled PSUM->SBUF.
  pass 2: high-half columns; partial reinjected via identity matmul; epilogue.
Interleaved ("dripped") phase-1 builds and two chunked h AllGathers keep the
Pool engine's gather-issue stream (the hard bottleneck at ~1m04ns per
128-edge column) running with almost no stalls across layer boundaries.
"""

import math
import sys
from contextlib import ExitStack

sys.path.insert(0, "/opt/trn_rl_repo")

import numpy as np

from concourse import bacc, bass, mybir
from concourse.bass_utils import run_bass_kernel_spmd
from concourse.masks import make_identity
from concourse.tile import TileContext

NCORES = 8
P = 128
C = 128
H = 4
D = 32
L = 2
OUT = 2
KV = 2 * C
THCAP = 16

f32 = mybir.dt.float32
bf16 = mybir.dt.bfloat16
i32 = mybir.dt.int32
i16 = mybir.dt.int16

LAST_RESULTS = None
_NC_CACHE = {}


def _ap(base, pattern):
    return bass.AP(base.tensor, base.offset, pattern)


def _build(SH, W, NPAD, Tlo, Thi, g_vals, has_bkv, has_bq, has_ba, has_bfc):
    nc = bacc.Bacc("TRN2", target_bir_lowering=False)
    W2 = W // 2
    SH2 = SH // 2
    NP2 = NPAD // 2
    Tw = [a + b for a, b in zip(Tlo, Thi)]
    TH = max(max(Tlo), max(Thi))
    assert TH <= THCAP
    ofs = [0]
    for t in Tw:
        ofs.append(ofs[-1] + t)
    CT = ofs[-1]

    xT = nc.dram_tensor("xT", [P, NPAD], bf16, kind="ExternalInput")
    hL0 = nc.dram_tensor("hL0", [P, SH], bf16, kind="ExternalInput")
    srcix_d = nc.dram_tensor("srcix", [P, CT], i32, kind="ExternalInput")
    dcol_d = nc.dram_tensor("dcol", [P, CT], f32, kind="ExternalInput")
    r01_d = nc.dram_tensor("r01", [W, P, 4], f32, kind="ExternalInput")
    Wkv_d = nc.dram_tensor("Wkv", [L, C, KV], bf16, kind="ExternalInput")
    Wq_d = nc.dram_tensor("Wq", [L, C, C], bf16, kind="ExternalInput")
    Wa_d = nc.dram_tensor("Wa", [L, C, C], bf16, kind="ExternalInput")
    Wfc_d = nc.dram_tensor("Wfc", [C, OUT], bf16, kind="ExternalInput")
    if has_bkv:
        bkv_d = nc.dram_tensor("bkv", [L, P, KV], bf16, kind="ExternalInput")
    if has_bq:
        bq_d = nc.dram_tensor("bq", [L, P, C], bf16, kind="ExternalInput")
    if has_ba:
        bag_d = nc.dram_tensor("bag", [L, C, 1], f32, kind="ExternalInput")
    if has_bfc:
        bfc_d = nc.dram_tensor("bfc", [P, OUT], f32, kind="ExternalInput")
    out_d = nc.dram_tensor("out", [SH, OUT], f32, kind="ExternalOutput")

    kvA_l = [nc.dram_tensor(f"kvA{l}", [NP2, KV], bf16) for l in range(L)]
    kvB_l = [nc.dram_tensor(f"kvB{l}", [NP2, KV], bf16) for l in range(L)]
    h1a = nc.dram_tensor("h1a", [P, SH2], bf16)
    h1b = nc.dram_tensor("h1b", [P, SH2], bf16)
    ag_a = nc.dram_tensor("ag_a", [NCORES, P, SH2], bf16, addr_space="Shared")
    ag_b = nc.dram_tensor("ag_b", [NCORES, P, SH2], bf16, addr_space="Shared")

    AFT = mybir.ActivationFunctionType
    ALU = mybir.AluOpType
    TP = TH * P

    with TileContext(nc) as tc, ExitStack() as ctx:
        cpool = ctx.enter_context(tc.tile_pool(name="consts", bufs=1))
        p1 = ctx.enter_context(tc.tile_pool(name="p1", bufs=3))
        pwin = ctx.enter_context(tc.tile_pool(name="pwin", bufs=2))
        pgath = ctx.enter_context(tc.tile_pool(name="pgath", bufs=6))
        pepi = ctx.enter_context(tc.tile_pool(name="pepi", bufs=2))
        ps = ctx.enter_context(tc.tile_pool(name="ps", bufs=2, space="PSUM"))

        ident = cpool.tile([P, P], bf16)
        make_identity(nc, ident[:])
        iota128 = cpool.tile([P, P], i16)
        nc.gpsimd.iota(iota128[:], pattern=[[1, P]], base=0, channel_multiplier=0)
        iotaF = cpool.tile([P, TP], i16)
        nc.gpsimd.iota(iotaF[:], pattern=[[1, TP]], base=0, channel_multiplier=0)
        iota128f = cpool.tile([P, P], f32)
        nc.scalar.activation(out=iota128f[:], in_=iota128[:], func=AFT.Copy)

        wkv_sb = cpool.tile([P, L * KV], bf16)
        wq_sb = cpool.tile([P, L * C], bf16)
        wa_sb = cpool.tile([P, L * C], bf16)
        wfc_sb = cpool.tile([P, OUT], bf16)
        for l in range(L):
            nc.sync.dma_start(out=wkv_sb[:, l * KV:(l + 1) * KV], in_=Wkv_d[l])
            nc.sync.dma_start(out=wq_sb[:, l * C:(l + 1) * C], in_=Wq_d[l])
            nc.sync.dma_start(out=wa_sb[:, l * C:(l + 1) * C], in_=Wa_d[l])
        nc.sync.dma_start(out=wfc_sb[:], in_=Wfc_d[:])
        if has_bkv:
            bkv_sb = cpool.tile([P, L * KV], bf16)
            for l in range(L):
                nc.sync.dma_start(out=bkv_sb[:, l * KV:(l + 1) * KV], in_=bkv_d[l])
        if has_bq:
            bq_sb = cpool.tile([P, L * C], bf16)
            for l in range(L):
                nc.sync.dma_start(out=bq_sb[:, l * C:(l + 1) * C], in_=bq_d[l])
        if has_ba:
            bag_sb = cpool.tile([P, L], f32)
            for l in range(L):
                nc.sync.dma_start(out=bag_sb[:, l:l + 1], in_=bag_d[l])
        if has_bfc:
            bfc_sb = cpool.tile([P, OUT], f32)
            nc.sync.dma_start(out=bfc_sb[:], in_=bfc_d[:])

        hloc = cpool.tile([P, SH], bf16)
        nc.sync.dma_start(out=hloc[:], in_=hL0[:, :])
        qloc = cpool.tile([P, SH], bf16)
        partial = cpool.tile([P, W * 132], bf16)

        def p1_group(l, half, kvX, s, g0):
            wkv_l = wkv_sb[:, l * KV:(l + 1) * KV]
            gl = min(4, W2 - g0)
            if l == 0:
                c0 = s * SH + half * SH2 + g0 * P
                src_ap = xT[:, c0:c0 + gl * P]
            else:
                agx = ag_a if half == 0 else ag_b
                src_ap = agx[s][:, g0 * P:(g0 + gl) * P]
            ht = p1.tile([P, 4 * P], bf16, tag="ht")
            nc.sync.dma_start(out=ht[:, :gl * P], in_=src_ap)
            kvb = p1.tile([P, 4 * KV], bf16, tag="kvb")
            for i in range(gl):
                pk = ps.tile([P, KV], f32, tag="pk")
                nc.tensor.matmul(pk[:], lhsT=ht[:, i * P:(i + 1) * P],
                                 rhs=wkv_l, start=True, stop=True)
                dst = kvb[:, i * KV:(i + 1) * KV]
                if has_bkv:
                    nc.vector.tensor_tensor(
                        out=dst, in0=pk[:],
                        in1=bkv_sb[:, l * KV:(l + 1) * KV], op=ALU.add)
                elif (g0 // 4 + i) % 2 == 0:
                    nc.scalar.activation(out=dst, in_=pk[:], func=AFT.Copy)
                else:
                    nc.vector.tensor_copy(dst, pk[:])
            row = s * SH2 + g0 * P
            base = kvX[row:row + P, :]
            nc.sync.dma_start(
                out=bass.AP(base.tensor, base.offset,
                            [[KV, P], [P * KV, gl], [1, KV]]),
                in_=kvb[:, :gl * KV])

        def groups_of(l, half, kvX):
            return [(l, half, kvX, s, g0)
                    for s in range(NCORES) for g0 in range(0, W2, 4)]

        def pass_cols(l, w, lo):
            """One window's work for one half (lo=True: pass 1, no epilogue)."""
            g = g_vals[l]
            wa_l = wa_sb[:, l * C:(l + 1) * C]
            tlo = Tlo[w]
            tn = tlo if lo else Thi[w]
            o = ofs[w] if lo else ofs[w] + tlo
            kvX = kvA_l[l] if lo else kvB_l[l]
            rr = (0, 1) if lo else (2, 3)
            if tn == 0 and lo:
                nc.vector.memset(partial[:, w * 132:(w + 1) * 132], 0)
                return
            tp = tn * P

            if tn:
                six = pgath.tile([P, TH], i32, tag="six")
                nc.sync.dma_start(out=six[:, :tn], in_=srcix_d[:, o:o + tn])
                dct = pgath.tile([P, TH], f32, tag="dct")
                nc.sync.dma_start(out=dct[:, :tn], in_=dcol_d[:, o:o + tn])
                r01 = pgath.tile([P, 4], f32, tag="r01")
                nc.sync.dma_start(out=r01[:], in_=r01_d[w])
                kva = pgath.tile([P, TH * KV], bf16, tag="kva")
                for t in range(tn):
                    nc.gpsimd.indirect_dma_start(
                        out=kva[:, t * KV:(t + 1) * KV], out_offset=None,
                        in_=kvX[:, :],
                        in_offset=bass.IndirectOffsetOnAxis(
                            ap=six[:, t:t + 1], axis=0))

                ST = pwin.tile([P, TP], bf16, tag="ST")
                nc.vector.tensor_scalar(
                    out=ST[:, :tp], in0=iotaF[:, :tp],
                    scalar1=r01[:, rr[1]:rr[1] + 1], scalar2=None, op0=ALU.is_lt)
                nc.vector.scalar_tensor_tensor(
                    out=ST[:, :tp], in0=iotaF[:, :tp],
                    scalar=r01[:, rr[0]:rr[0] + 1],
                    in1=ST[:, :tp], op0=ALU.is_ge, op1=ALU.mult)
                S = pwin.tile([P, TP], bf16, tag="S")
                nc.vector.tensor_tensor(
                    out=S[:, :tp].rearrange("p (t n) -> p t n", n=P),
                    in0=dct[:, :tn].to_broadcast([P, tn, P]),
                    in1=_ap(iota128f[:], [[P, P], [0, tn], [1, P]]),
                    op=ALU.is_equal)

            ags = ps.tile([P, 132], f32, tag="ags")
            if tn:
                kva3 = kva[:].rearrange("p (t c) -> p t c", c=KV)
                qw = qloc[:, w * C:(w + 1) * C]
                qsb = pwin.tile([P, TP], bf16, tag="qsb")
                t0 = 0
                while t0 < tn:
                    gl = min(4, tn - t0)
                    psq = ps.tile([P, 512], f32, tag="psq")
                    for i in range(gl):
                        t = t0 + i
                        nc.tensor.matmul(psq[:, i * P:(i + 1) * P],
                                         lhsT=ST[:, t * P:(t + 1) * P],
                                         rhs=qw, start=True, stop=True)
                    nc.scalar.activation(out=qsb[:, t0 * P:(t0 + gl) * P],
                                         in_=psq[:, :gl * P], func=AFT.Copy)
                    t0 += gl
                prod = pwin.tile([P, TP], bf16, tag="prod")
                pv = prod[:].rearrange("p (t h d) -> p t h d", h=H, d=D)
                nc.vector.tensor_tensor(
                    out=pv[:, :tn],
                    in0=qsb[:, :tp].rearrange("p (t h d) -> p t h d", h=H, d=D),
                    in1=kva3[:, :tn, 0:C].rearrange("p t (h d) -> p t h d", d=D),
                    op=ALU.mult)
                f1 = pwin.tile([P, TH * H * 16], bf16, tag="f1")
                f1v = f1[:].rearrange("p (t h d) -> p t h d", h=H, d=16)
                nc.vector.tensor_tensor(out=f1v[:, :tn], in0=pv[:, :tn, :, 0:16],
                                        in1=pv[:, :tn, :, 16:32], op=ALU.add)
                f2 = pwin.tile([P, TH * H * 8], bf16, tag="f2")
                f2v = f2[:].rearrange("p (t h d) -> p t h d", h=H, d=8)
                nc.vector.tensor_tensor(out=f2v[:, :tn], in0=f1v[:, :tn, :, 0:8],
                                        in1=f1v[:, :tn, :, 8:16], op=ALU.add)
                f3 = pwin.tile([P, TH * H * 4], bf16, tag="f3")
                f3v = f3[:].rearrange("p (t h d) -> p t h d", h=H, d=4)
                nc.vector.tensor_tensor(out=f3v[:, :tn], in0=f2v[:, :tn, :, 0:4],
                                        in1=f2v[:, :tn, :, 4:8], op=ALU.add)
                f4 = pwin.tile([P, TH * H * 2], bf16, tag="f4")
                f4v = f4[:].rearrange("p (t h d) -> p t h d", h=H, d=2)
                nc.vector.tensor_tensor(out=f4v[:, :tn], in0=f3v[:, :tn, :, 0:2],
                                        in1=f3v[:, :tn, :, 2:4], op=ALU.add)
                alpha = pwin.tile([P, TH * H], bf16, tag="alpha")
                av = alpha[:].rearrange("p (t h) -> p t h", h=H)
                nc.vector.tensor_tensor(out=av[:, :tn],
                                        in0=f4v[:, :tn, :, 0:1],
                                        in1=f4v[:, :tn, :, 1:2], op=ALU.add)
                msg = pwin.tile([P, TH * 132], bf16, tag="msg")
                mv = msg[:].rearrange("p (t c) -> p t c", c=132)
                nc.scalar.activation(out=mv[:, :tn, C:C + 4],
                                     in_=av[:, :tn], func=AFT.Exp)
                ez_ap = _ap(msg[:, C:C + 4],
                            [[TH * 132, P], [132, tn], [0, D], [1, H]])
                nc.vector.tensor_tensor(
                    out=mv[:, :tn, 0:C].rearrange("p t (d h) -> p t d h", h=H),
                    in0=kva3[:, :tn, C:KV].rearrange("p t (d h) -> p t d h", h=H),
                    in1=ez_ap,
                    op=ALU.mult)
            if lo:
                if tn:
                    for t in range(tn):
                        nc.tensor.matmul(ags[:, 0:132],
                                         lhsT=S[:, t * P:(t + 1) * P],
                                         rhs=msg[:, t * 132:(t + 1) * 132],
                                         start=(t == 0), stop=(t == tn - 1),
                                         skip_group_check=True)
                    nc.vector.tensor_copy(partial[:, w * 132:(w + 1) * 132],
                                          ags[:, 0:132])
                return
            # pass 2: reinject partial, accumulate hi cols, epilogue
            nc.tensor.matmul(ags[:, 0:132], lhsT=ident[:],
                             rhs=partial[:, w * 132:(w + 1) * 132],
                             start=True, stop=(tn == 0), skip_group_check=True)
            for t in range(tn):
                nc.tensor.matmul(ags[:, 0:132],
                                 lhsT=S[:, t * P:(t + 1) * P],
                                 rhs=msg[:, t * 132:(t + 1) * 132],
                                 start=False, stop=(t == tn - 1),
                                 skip_group_check=True)

            den = pepi.tile([P, 4], f32, tag="den")
            nc.vector.tensor_scalar_max(den[:], ags[:, C:C + 4], 1e-30)
            rec = pepi.tile([P, 4], f32, tag="rec")
            nc.vector.reciprocal(rec[:], den[:])
            aggn = pepi.tile([P, C], bf16, tag="aggn")
            nc.vector.tensor_tensor(
                out=aggn[:].rearrange("p (d h) -> p d h", h=H),
                in0=ags[:, 0:C].rearrange("p (d h) -> p d h", h=H),
                in1=_ap(rec[:], [[4, P], [0, D], [1, H]]),
                op=ALU.mult)
            gact = pepi.tile([P, C], bf16, tag="gact")
            nc.scalar.activation(out=gact[:], in_=aggn[:], func=AFT.Gelu)
            gt = ps.tile([P, 2 * P], bf16, tag="epi")
            nc.tensor.transpose(gt[:, :P], gact[:], ident[:])
            gts = pepi.tile([P, P], bf16, tag="gts")
            nc.scalar.activation(out=gts[:], in_=gt[:, :P], func=AFT.Copy)
            op_ = ps.tile([P, P], f32, tag="epi")
            nc.tensor.matmul(op_[:], lhsT=wa_l, rhs=gts[:],
                             start=True, stop=True)
            hsl = hloc[:, w * P:(w + 1) * P]
            nc.vector.scalar_tensor_tensor(
                out=hsl, in0=hsl, scalar=float(1.0 - g), in1=op_[:],
                op0=ALU.mult, op1=ALU.add)
            if has_ba:
                nc.vector.tensor_tensor(
                    out=hsl, in0=hsl,
                    in1=bag_sb[:, l:l + 1].to_broadcast([P, P]), op=ALU.add)
            if l == 0:
                if w < W2:
                    nc.sync.dma_start(out=h1a[:, w * P:(w + 1) * P], in_=hsl)
                else:
                    nc.sync.dma_start(
                        out=h1b[:, (w - W2) * P:(w - W2 + 1) * P], in_=hsl)
                if w == min(W2 + 5, W - 1):
                    nc.gpsimd.collective_compute(
                        "AllGather", ALU.bypass,
                        replica_groups=[list(range(NCORES))],
                        ins=[h1a[:]], outs=[ag_a[:]])
                if w == W - 1:
                    nc.gpsimd.collective_compute(
                        "AllGather", ALU.bypass,
                        replica_groups=[list(range(NCORES))],
                        ins=[h1b[:]], outs=[ag_b[:]])
            else:
                po = ps.tile([P, OUT], f32, tag="epi")
                nc.tensor.matmul(po[:], lhsT=hsl, rhs=wfc_sb[:],
                                 start=True, stop=True)
                ob = pepi.tile([P, OUT], f32, tag="ob")
                if has_bfc:
                    nc.vector.tensor_tensor(out=ob[:], in0=po[:],
                                            in1=bfc_sb[:], op=ALU.add)
                else:
                    nc.scalar.activation(out=ob[:], in_=po[:], func=AFT.Copy)
                nc.sync.dma_start(out=out_d[w * P:(w + 1) * P, :], in_=ob[:])

        # ---------------- layer schedule with dripped phase-1 --------------
        drip = []

        def drip_emit(n):
            for _ in range(min(n, len(drip))):
                p1_group(*drip.pop(0))

        for l in range(L):
            wq_l = wq_sb[:, l * C:(l + 1) * C]
            if l == 0:
                for args in groups_of(0, 0, kvA_l[0]):
                    p1_group(*args)            # kvA0 eager (first dependency)
                drip = list(groups_of(0, 1, kvB_l[0]))
            # local q for this layer (reads hloc; cheap)
            for w in range(W):
                pq = ps.tile([P, KV], f32, tag="pk")
                nc.tensor.matmul(pq[:, :C], lhsT=hloc[:, w * P:(w + 1) * P],
                                 rhs=wq_l, start=True, stop=True)
                if has_bq:
                    nc.vector.tensor_tensor(
                        out=qloc[:, w * C:(w + 1) * C], in0=pq[:, :C],
                        in1=bq_sb[:, l * C:(l + 1) * C], op=ALU.add)
                else:
                    nc.scalar.activation(out=qloc[:, w * C:(w + 1) * C],
                                         in_=pq[:, :C], func=AFT.Copy)
            # pass 1 (low halves), dripping deferred phase-1 builds
            for w in range(W):
                pass_cols(l, w, lo=True)
                if l == 0 or w >= 34:
                    drip_emit(5)
            drip_emit(10 ** 9)
            # pass 2 (high halves) + epilogues; during l==0 drip layer-1 kvA
            if l == 0:
                drip = list(groups_of(1, 0, kvA_l[1]))
            for w in range(W):
                pass_cols(l, w, lo=False)
                if l == 0 and w >= W2 + 30:
                    drip_emit(6)
            if l == 0:
                drip_emit(10 ** 9)
                drip = list(groups_of(1, 1, kvB_l[1]))

    nc.compile()
    return nc


def _prep_host(x, edge_index, Wk, bk, Wq, bq, Wv, bv, a_rel, m_rel, p_rel,
               Wa, ba, skip, Wfc, bfc):
    import ml_dtypes
    bfd = ml_dtypes.bfloat16
    N = x.shape[0]
    SH = int(math.ceil(N / NCORES / P / 2)) * P * 2
    W = SH // P
    W2 = W // 2
    SH2 = SH // 2
    NPAD = NCORES * SH

    Wk_eff = np.einsum("lchd,lhde->lche", Wk.reshape(L, C, H, D),
                       a_rel, optimize=True).reshape(L, C, C)
    bk_eff = np.einsum("lhd,lhde->lhe", bk.reshape(L, H, D), a_rel).reshape(L, C)
    Wv_eff = np.einsum("lchd,lhde->lche", Wv.reshape(L, C, H, D),
                       m_rel, optimize=True).reshape(L, C, C)
    bv_eff = np.einsum("lhd,lhde->lhe", bv.reshape(L, H, D), m_rel).reshape(L, C)
    scale = (p_rel / np.sqrt(D)).astype(np.float32)
    Wq_eff = (Wq.reshape(L, C, H, D) * scale[:, None, :, None]).reshape(L, C, C)
    bq_eff = (bq.reshape(L, H, D) * scale[:, :, None]).reshape(L, C)
    g_vals = [float(1.0 / (1.0 + np.exp(-skip[l]))) for l in range(L)]

    dh = (np.arange(C) % H) * D + np.arange(C) // H
    Wv2 = Wv_eff[:, :, dh]
    bv2 = bv_eff[:, dh]
    Wkv = np.concatenate([Wk_eff, Wv2], axis=2)
    bkv = np.concatenate([bk_eff, bv2], axis=1)
    Wa_eff = np.stack([g_vals[l] * Wa[l][dh, :] for l in range(L)])
    bag = np.stack([g_vals[l] * ba[l] for l in range(L)])

    src = np.asarray(edge_index[0], np.int64)
    dst = np.asarray(edge_index[1], np.int64)
    core = dst // SH

    percore = []
    Tlo_all = np.zeros([NCORES, W], np.int64)
    Thi_all = np.zeros([NCORES, W], np.int64)
    for m in range(NCORES):
        sel = core == m
        d = (dst[sel] - m * SH).astype(np.int32)
        s_ = src[sel].astype(np.int32)
        half = ((s_ % SH) >= SH2).astype(np.int32)
        o = np.lexsort((d, half, d >> 7))
        d = d[o]
        s_ = s_[o]
        half = half[o]
        win = d >> 7
        grp = win * 2 + half
        cnt = np.bincount(grp, minlength=2 * W)
        Tlo_all[m] = (cnt[0::2] + P - 1) // P
        Thi_all[m] = (cnt[1::2] + P - 1) // P
        percore.append((d, s_, half, win, grp, cnt))
    Tlo = tuple(int(t) for t in Tlo_all.max(axis=0))
    Thi = tuple(int(t) for t in Thi_all.max(axis=0))
    Tw = np.array(Tlo) + np.array(Thi)
    ofs = np.zeros(W + 1, np.int64)
    ofs[1:] = np.cumsum(Tw)
    CT = int(ofs[-1])

    xTb = np.zeros([P, NPAD], bfd)
    xTb[:, :N] = np.ascontiguousarray(x.T).astype(bfd)

    in_maps = []
    for m in range(NCORES):
        d, s_, half, win, grp, cnt = percore[m]
        srcix = np.zeros([P, CT], np.int32)
        dcol = np.full([P, CT], -1, np.float32)
        lo = half == 0
        ncnt_lo = np.bincount(d[lo], minlength=SH).reshape(W, P)
        ncnt_hi = np.bincount(d[~lo], minlength=SH).reshape(W, P)
        r1a = np.cumsum(ncnt_lo, axis=1).astype(np.float32)
        r0a = r1a - ncnt_lo
        r1b = np.cumsum(ncnt_hi, axis=1).astype(np.float32)
        r0b = r1b - ncnt_hi
        r01 = np.stack([r0a, r1a, r0b, r1b], axis=2).astype(np.float32)
        if len(d):
            starts = np.zeros(2 * W, np.int64)
            starts[1:] = np.cumsum(cnt)[:-1]
            j = np.arange(len(d)) - starts[grp]
            t = (j >> 7).astype(np.int64)
            p = (j & 127).astype(np.int64)
            cbase = np.where(half == 0, ofs[win], ofs[win] + np.array(Tlo)[win])
            col = cbase + t
            shard = s_ // SH
            iloc = s_ % SH
            row = shard * SH2 + np.where(half == 0, iloc, iloc - SH2)
            srcix[p, col] = row
            dcol[p, col] = (d & 127).astype(np.float32)
        im = {
            "xT": xTb,
            "hL0": np.ascontiguousarray(xTb[:, m * SH:(m + 1) * SH]),
            "srcix": srcix,
            "dcol": dcol,
            "r01": np.ascontiguousarray(r01),
            "Wkv": np.ascontiguousarray(Wkv).astype(bfd),
            "Wq": np.ascontiguousarray(Wq_eff).astype(bfd),
            "Wa": np.ascontiguousarray(Wa_eff).astype(bfd),
            "Wfc": np.ascontiguousarray(Wfc).astype(bfd),
        }
        flags = dict(
            has_bkv=bool(np.any(bkv != 0)),
            has_bq=bool(np.any(bq_eff != 0)),
            has_ba=bool(np.any(bag != 0)),
            has_bfc=bool(np.any(bfc != 0)),
        )
        if flags["has_bkv"]:
            im["bkv"] = np.ascontiguousarray(
                np.broadcast_to(bkv[:, None, :], (L, P, KV))).astype(bfd)
        if flags["has_bq"]:
            im["bq"] = np.ascontiguousarray(
                np.broadcast_to(bq_eff[:, None, :], (L, P, C))).astype(bfd)
        if flags["has_ba"]:
            im["bag"] = np.ascontiguousarray(bag[:, dh, None], dtype=np.float32)
        if flags["has_bfc"]:
            im["bfc"] = np.ascontiguousarray(
                np.broadcast_to(bfc[None, :], (P, OUT)), dtype=np.float32)
        in_maps.append(im)

    return SH, W, NPAD, Tlo, Thi, g_vals, in_maps, flags


def kernel(x, edge_index, Wk, bk, Wq, bq, Wv, bv, a_rel, m_rel, p_rel,
           Wa, ba, skip, Wfc, bfc, trace=False):
    global LAST_RESULTS
    x = np.asarray(x, np.float32)
    args = [np.asarray(a, np.float32) for a in
            (Wk, bk, Wq, bq, Wv, bv, a_rel, m_rel, p_rel, Wa, ba, skip, Wfc, bfc)]
    N = x.shape[0]

    SH, W, NPAD, Tlo, Thi, g_vals, in_maps, flags = _prep_host(
        x, edge_index, *args)
    key = (SH, W, NPAD, Tlo, Thi, tuple(g_vals), tuple(sorted(flags.items())))
    nc = _NC_CACHE.get(key)
    if nc is None:
        nc = _build(SH, W, NPAD, Tlo, Thi, g_vals, **flags)
        _NC_CACHE[key] = nc
    try:
        res = run_bass_kernel_spmd(nc, in_maps, list(range(NCORES)), trace=trace)
    except ModuleNotFoundError:
        res = run_bass_kernel_spmd(nc, in_maps, list(range(NCORES)), trace=False)
    LAST_RESULTS = res

    out = np.empty([N, OUT], np.float32)
    for m in range(NCORES):
        lo = m * SH
        hi = min(N, lo + SH)
        if hi > lo:
            out[lo:hi] = res.results[m]["out"][:hi - lo]
    return out


# revision 3
# speedup vs baseline: 1.4945x; 1.4945x over previous
"""HGT on 8 trn2 NeuronCores — bf16, two-pass phase-2 pipeline (v5).

Dst-sharded node partition; kv tables split by src node half (kvA/kvB,
per-layer tensors). Phase 2 runs in two passes over all windows:
  pass 1: low-half columns only; partial (agg|ez-sum) spilled PSUM->SBUF.
  pass 2: high-half columns; partial reinjected via identity matmul; epilogue.
Interleaved ("dripped") phase-1 builds and two chunked h AllGathers keep the
Pool engine's gather-issue stream (the hard bottleneck at ~1m04ns per
128-edge column) running with almost no stalls across layer boundaries.
"""

import math
import sys
from contextlib import ExitStack

sys.path.insert(0, "/opt/trn_rl_repo")

import numpy as np

from concourse import bacc, bass, mybir
from concourse.bass_utils import run_bass_kernel_spmd
from concourse.masks import make_identity
from concourse.tile import TileContext

NCORES = 8
P = 128
C = 128
H = 4
D = 32
L = 2
OUT = 2
KV = 2 * C
THCAP = 16

f32 = mybir.dt.float32
bf16 = mybir.dt.bfloat16
i32 = mybir.dt.int32
i16 = mybir.dt.int16

LAST_RESULTS = None
_NC_CACHE = {}


def _ap(base, pattern):
    return bass.AP(base.tensor, base.offset, pattern)


def _build(SH, W, NPAD, Tlo, Thi, g_vals, has_bkv, has_bq, has_ba, has_bfc):
    nc = bacc.Bacc("TRN2", target_bir_lowering=False)
    W2 = W // 2
    SH2 = SH // 2
    NP2 = NPAD // 2
    Tw = [a + b for a, b in zip(Tlo, Thi)]
    TH = max(max(Tlo), max(Thi))
    assert TH <= THCAP
    ofs = [0]
    for t in Tw:
        ofs.append(ofs[-1] + t)
    CT = ofs[-1]

    xT = nc.dram_tensor("xT", [P, NPAD], bf16, kind="ExternalInput")
    hL0 = nc.dram_tensor("hL0", [P, SH], bf16, kind="ExternalInput")
    srcix_d = nc.dram_tensor("srcix", [P, CT], i32, kind="ExternalInput")
    dcol_d = nc.dram_tensor("dcol", [P, CT], f32, kind="ExternalInput")
    r01_d = nc.dram_tensor("r01", [W, P, 4], f32, kind="ExternalInput")
    Wkv_d = nc.dram_tensor("Wkv", [L, C, KV], bf16, kind="ExternalInput")
    Wq_d = nc.dram_tensor("Wq", [L, C, C], bf16, kind="ExternalInput")
    Wa_d = nc.dram_tensor("Wa", [L, C, C], bf16, kind="ExternalInput")
    Wfc_d = nc.dram_tensor("Wfc", [C, OUT], bf16, kind="ExternalInput")
    if has_bkv:
        bkv_d = nc.dram_tensor("bkv", [L, P, KV], bf16, kind="ExternalInput")
    if has_bq:
        bq_d = nc.dram_tensor("bq", [L, P, C], bf16, kind="ExternalInput")
    if has_ba:
        bag_d = nc.dram_tensor("bag", [L, C, 1], f32, kind="ExternalInput")
    if has_bfc:
        bfc_d = nc.dram_tensor("bfc", [P, OUT], f32, kind="ExternalInput")
    out_d = nc.dram_tensor("out", [SH, OUT], f32, kind="ExternalOutput")

    kvA_l = [nc.dram_tensor(f"kvA{l}", [NP2, KV], bf16) for l in range(L)]
    kvB_l = [nc.dram_tensor(f"kvB{l}", [NP2, KV], bf16) for l in range(L)]
    h1a = nc.dram_tensor("h1a", [P, SH2], bf16)
    h1b = nc.dram_tensor("h1b", [P, SH2], bf16)
    ag_a = nc.dram_tensor("ag_a", [NCORES, P, SH2], bf16, addr_space="Shared")
    ag_b = nc.dram_tensor("ag_b", [NCORES, P, SH2], bf16, addr_space="Shared")

    AFT = mybir.ActivationFunctionType
    ALU = mybir.AluOpType
    TP = TH * P

    with TileContext(nc) as tc, ExitStack() as ctx:
        cpool = ctx.enter_context(tc.tile_pool(name="consts", bufs=1))
        p1 = ctx.enter_context(tc.tile_pool(name="p1", bufs=3))
        pwin = ctx.enter_context(tc.tile_pool(name="pwin", bufs=3))
        pgath = ctx.enter_context(tc.tile_pool(name="pgath", bufs=8))
        pepi = ctx.enter_context(tc.tile_pool(name="pepi", bufs=2))
        ps = ctx.enter_context(tc.tile_pool(name="ps", bufs=2, space="PSUM"))

        ident = cpool.tile([P, P], bf16)
        make_identity(nc, ident[:])
        iota128 = cpool.tile([P, P], i16)
        nc.gpsimd.iota(iota128[:], pattern=[[1, P]], base=0, channel_multiplier=0)
        iotaF = cpool.tile([P, TP], i16)
        nc.gpsimd.iota(iotaF[:], pattern=[[1, TP]], base=0, channel_multiplier=0)
        iota128f = cpool.tile([P, P], f32)
        nc.scalar.activation(out=iota128f[:], in_=iota128[:], func=AFT.Copy)

        wkv_sb = cpool.tile([P, L * KV], bf16)
        wq_sb = cpool.tile([P, L * C], bf16)
        wa_sb = cpool.tile([P, L * C], bf16)
        wfc_sb = cpool.tile([P, OUT], bf16)
        for l in range(L):
            nc.sync.dma_start(out=wkv_sb[:, l * KV:(l + 1) * KV], in_=Wkv_d[l])
            nc.sync.dma_start(out=wq_sb[:, l * C:(l + 1) * C], in_=Wq_d[l])
            nc.sync.dma_start(out=wa_sb[:, l * C:(l + 1) * C], in_=Wa_d[l])
        nc.sync.dma_start(out=wfc_sb[:], in_=Wfc_d[:])
        if has_bkv:
            bkv_sb = cpool.tile([P, L * KV], bf16)
            for l in range(L):
                nc.sync.dma_start(out=bkv_sb[:, l * KV:(l + 1) * KV], in_=bkv_d[l])
        if has_bq:
            bq_sb = cpool.tile([P, L * C], bf16)
            for l in range(L):
                nc.sync.dma_start(out=bq_sb[:, l * C:(l + 1) * C], in_=bq_d[l])
        if has_ba:
            bag_sb = cpool.tile([P, L], f32)
            for l in range(L):
                nc.sync.dma_start(out=bag_sb[:, l:l + 1], in_=bag_d[l])
        if has_bfc:
            bfc_sb = cpool.tile([P, OUT], f32)
            nc.sync.dma_start(out=bfc_sb[:], in_=bfc_d[:])

        hloc = cpool.tile([P, SH], bf16)
        nc.sync.dma_start(out=hloc[:], in_=hL0[:, :])
        qloc = cpool.tile([P, SH], bf16)
        partial = cpool.tile([P, W * 132], bf16)

        def p1_group(l, half, kvX, s, g0):
            wkv_l = wkv_sb[:, l * KV:(l + 1) * KV]
            gl = min(4, W2 - g0)
            if l == 0:
                c0 = s * SH + half * SH2 + g0 * P
                src_ap = xT[:, c0:c0 + gl * P]
            else:
                agx = ag_a if half == 0 else ag_b
                src_ap = agx[s][:, g0 * P:(g0 + gl) * P]
            ht = p1.tile([P, 4 * P], bf16, tag="ht")
            nc.sync.dma_start(out=ht[:, :gl * P], in_=src_ap)
            kvb = p1.tile([P, 4 * KV], bf16, tag="kvb")
            for i in range(gl):
                pk = ps.tile([P, KV], f32, tag="pk")
                nc.tensor.matmul(pk[:], lhsT=ht[:, i * P:(i + 1) * P],
                                 rhs=wkv_l, start=True, stop=True)
                dst = kvb[:, i * KV:(i + 1) * KV]
                if has_bkv:
                    nc.vector.tensor_tensor(
                        out=dst, in0=pk[:],
                        in1=bkv_sb[:, l * KV:(l + 1) * KV], op=ALU.add)
                elif (g0 // 4 + i) % 2 == 0:
                    nc.scalar.activation(out=dst, in_=pk[:], func=AFT.Copy)
                else:
                    nc.vector.tensor_copy(dst, pk[:])
            row = s * SH2 + g0 * P
            base = kvX[row:row + P, :]
            nc.sync.dma_start(
                out=bass.AP(base.tensor, base.offset,
                            [[KV, P], [P * KV, gl], [1, KV]]),
                in_=kvb[:, :gl * KV])

        def groups_of(l, half, kvX):
            return [(l, half, kvX, s, g0)
                    for s in range(NCORES) for g0 in range(0, W2, 4)]

        def stage_a(l, w, lo):
            """Loads, gathers, masks, q-select for one window half."""
            tlo = Tlo[w]
            tn = tlo if lo else Thi[w]
            o = ofs[w] if lo else ofs[w] + tlo
            kvX = kvA_l[l] if lo else kvB_l[l]
            rr = (0, 1) if lo else (2, 3)
            if tn == 0 and lo:
                nc.vector.memset(partial[:, w * 132:(w + 1) * 132], 0)
                return None
            tp = tn * P
            ctx = dict(l=l, w=w, lo=lo, tn=tn)
            if tn:
                six = pgath.tile([P, TH], i32, tag="six")
                nc.sync.dma_start(out=six[:, :tn], in_=srcix_d[:, o:o + tn])
                dct = pgath.tile([P, TH], f32, tag="dct")
                nc.sync.dma_start(out=dct[:, :tn], in_=dcol_d[:, o:o + tn])
                r01 = pgath.tile([P, 4], f32, tag="r01")
                nc.sync.dma_start(out=r01[:], in_=r01_d[w])
                kva = pgath.tile([P, TH * KV], bf16, tag="kva")
                for t in range(tn):
                    nc.gpsimd.indirect_dma_start(
                        out=kva[:, t * KV:(t + 1) * KV], out_offset=None,
                        in_=kvX[:, :],
                        in_offset=bass.IndirectOffsetOnAxis(
                            ap=six[:, t:t + 1], axis=0))
                ST = pwin.tile([P, TP], bf16, tag="ST")
                nc.vector.tensor_scalar(
                    out=ST[:, :tp], in0=iotaF[:, :tp],
                    scalar1=r01[:, rr[1]:rr[1] + 1], scalar2=None, op0=ALU.is_lt)
                nc.vector.scalar_tensor_tensor(
                    out=ST[:, :tp], in0=iotaF[:, :tp],
                    scalar=r01[:, rr[0]:rr[0] + 1],
                    in1=ST[:, :tp], op0=ALU.is_ge, op1=ALU.mult)
                S = pwin.tile([P, TP], bf16, tag="S")
                nc.vector.tensor_tensor(
                    out=S[:, :tp].rearrange("p (t n) -> p t n", n=P),
                    in0=dct[:, :tn].to_broadcast([P, tn, P]),
                    in1=_ap(iota128f[:], [[P, P], [0, tn], [1, P]]),
                    op=ALU.is_equal)
                qw = qloc[:, w * C:(w + 1) * C]
                qsb = pwin.tile([P, TP], bf16, tag="qsb")
                t0 = 0
                while t0 < tn:
                    gl = min(4, tn - t0)
                    psq = ps.tile([P, 512], f32, tag="psq")
                    for i in range(gl):
                        t = t0 + i
                        nc.tensor.matmul(psq[:, i * P:(i + 1) * P],
                                         lhsT=ST[:, t * P:(t + 1) * P],
                                         rhs=qw, start=True, stop=True)
                    nc.scalar.activation(out=qsb[:, t0 * P:(t0 + gl) * P],
                                         in_=psq[:, :gl * P], func=AFT.Copy)
                    t0 += gl
                ctx.update(kva=kva, S=S, qsb=qsb)
            return ctx

        def stage_b(ctx):
            """Products, softmax pieces, scatter accumulation, epilogue."""
            if ctx is None:
                return
            l, w, lo, tn = ctx["l"], ctx["w"], ctx["lo"], ctx["tn"]
            g = g_vals[l]
            wa_l = wa_sb[:, l * C:(l + 1) * C]
            ags = ps.tile([P, 132], f32, tag="ags")
            if tn:
                kva, S, qsb = ctx["kva"], ctx["S"], ctx["qsb"]
                tp = tn * P
                kva3 = kva[:].rearrange("p (t c) -> p t c", c=KV)
                prod = pwin.tile([P, TP], bf16, tag="prod")
                pv = prod[:].rearrange("p (t h d) -> p t h d", h=H, d=D)
                nc.vector.tensor_tensor(
                    out=pv[:, :tn],
                    in0=qsb[:, :tp].rearrange("p (t h d) -> p t h d", h=H, d=D),
                    in1=kva3[:, :tn, 0:C].rearrange("p t (h d) -> p t h d", d=D),
                    op=ALU.mult)
                f1 = pwin.tile([P, TH * H * 16], bf16, tag="f1")
                f1v = f1[:].rearrange("p (t h d) -> p t h d", h=H, d=16)
                nc.vector.tensor_tensor(out=f1v[:, :tn], in0=pv[:, :tn, :, 0:16],
                                        in1=pv[:, :tn, :, 16:32], op=ALU.add)
                f2 = pwin.tile([P, TH * H * 8], bf16, tag="f2")
                f2v = f2[:].rearrange("p (t h d) -> p t h d", h=H, d=8)
                nc.vector.tensor_tensor(out=f2v[:, :tn], in0=f1v[:, :tn, :, 0:8],
                                        in1=f1v[:, :tn, :, 8:16], op=ALU.add)
                f3 = pwin.tile([P, TH * H * 4], bf16, tag="f3")
                f3v = f3[:].rearrange("p (t h d) -> p t h d", h=H, d=4)
                nc.vector.tensor_tensor(out=f3v[:, :tn], in0=f2v[:, :tn, :, 0:4],
                                        in1=f2v[:, :tn, :, 4:8], op=ALU.add)
                f4 = pwin.tile([P, TH * H * 2], bf16, tag="f4")
                f4v = f4[:].rearrange("p (t h d) -> p t h d", h=H, d=2)
                nc.vector.tensor_tensor(out=f4v[:, :tn], in0=f3v[:, :tn, :, 0:2],
                                        in1=f3v[:, :tn, :, 2:4], op=ALU.add)
                alpha = pwin.tile([P, TH * H], bf16, tag="alpha")
                av = alpha[:].rearrange("p (t h) -> p t h", h=H)
                nc.vector.tensor_tensor(out=av[:, :tn],
                                        in0=f4v[:, :tn, :, 0:1],
                                        in1=f4v[:, :tn, :, 1:2], op=ALU.add)
                msg = pwin.tile([P, TH * 132], bf16, tag="msg")
                mv = msg[:].rearrange("p (t c) -> p t c", c=132)
                nc.scalar.activation(out=mv[:, :tn, C:C + 4],
                                     in_=av[:, :tn], func=AFT.Exp)
                ez_ap = _ap(msg[:, C:C + 4],
                            [[TH * 132, P], [132, tn], [0, D], [1, H]])
                nc.vector.tensor_tensor(
                    out=mv[:, :tn, 0:C].rearrange("p t (d h) -> p t d h", h=H),
                    in0=kva3[:, :tn, C:KV].rearrange("p t (d h) -> p t d h", h=H),
                    in1=ez_ap,
                    op=ALU.mult)
            if lo:
                if tn:
                    for t in range(tn):
                        nc.tensor.matmul(ags[:, 0:132],
                                         lhsT=S[:, t * P:(t + 1) * P],
                                         rhs=msg[:, t * 132:(t + 1) * 132],
                                         start=(t == 0), stop=(t == tn - 1),
                                         skip_group_check=True)
                    nc.vector.tensor_copy(partial[:, w * 132:(w + 1) * 132],
                                          ags[:, 0:132])
                return
            nc.tensor.matmul(ags[:, 0:132], lhsT=ident[:],
                             rhs=partial[:, w * 132:(w + 1) * 132],
                             start=True, stop=(tn == 0), skip_group_check=True)
            for t in range(tn):
                nc.tensor.matmul(ags[:, 0:132],
                                 lhsT=S[:, t * P:(t + 1) * P],
                                 rhs=msg[:, t * 132:(t + 1) * 132],
                                 start=False, stop=(t == tn - 1),
                                 skip_group_check=True)

            den = pepi.tile([P, 4], f32, tag="den")
            nc.vector.tensor_scalar_max(den[:], ags[:, C:C + 4], 1e-30)
            rec = pepi.tile([P, 4], f32, tag="rec")
            nc.vector.reciprocal(rec[:], den[:])
            aggn = pepi.tile([P, C], bf16, tag="aggn")
            nc.vector.tensor_tensor(
                out=aggn[:].rearrange("p (d h) -> p d h", h=H),
                in0=ags[:, 0:C].rearrange("p (d h) -> p d h", h=H),
                in1=_ap(rec[:], [[4, P], [0, D], [1, H]]),
                op=ALU.mult)
            gact = pepi.tile([P, C], bf16, tag="gact")
            nc.scalar.activation(out=gact[:], in_=aggn[:], func=AFT.Gelu)
            gt = ps.tile([P, 2 * P], bf16, tag="epi")
            nc.tensor.transpose(gt[:, :P], gact[:], ident[:])
            gts = pepi.tile([P, P], bf16, tag="gts")
            nc.scalar.activation(out=gts[:], in_=gt[:, :P], func=AFT.Copy)
            op_ = ps.tile([P, P], f32, tag="epi")
            nc.tensor.matmul(op_[:], lhsT=wa_l, rhs=gts[:],
                             start=True, stop=True)
            hsl = hloc[:, w * P:(w + 1) * P]
            nc.vector.scalar_tensor_tensor(
                out=hsl, in0=hsl, scalar=float(1.0 - g), in1=op_[:],
                op0=ALU.mult, op1=ALU.add)
            if has_ba:
                nc.vector.tensor_tensor(
                    out=hsl, in0=hsl,
                    in1=bag_sb[:, l:l + 1].to_broadcast([P, P]), op=ALU.add)
            if l == 0:
                if w < W2:
                    nc.sync.dma_start(out=h1a[:, w * P:(w + 1) * P], in_=hsl)
                else:
                    nc.sync.dma_start(
                        out=h1b[:, (w - W2) * P:(w - W2 + 1) * P], in_=hsl)
                if w == min(W2 + 5, W - 1):
                    nc.gpsimd.collective_compute(
                        "AllGather", ALU.bypass,
                        replica_groups=[list(range(NCORES))],
                        ins=[h1a[:]], outs=[ag_a[:]])
                if w == W - 1:
                    nc.gpsimd.collective_compute(
                        "AllGather", ALU.bypass,
                        replica_groups=[list(range(NCORES))],
                        ins=[h1b[:]], outs=[ag_b[:]])
            else:
                po = ps.tile([P, OUT], f32, tag="epi")
                nc.tensor.matmul(po[:], lhsT=hsl, rhs=wfc_sb[:],
                                 start=True, stop=True)
                ob = pepi.tile([P, OUT], f32, tag="ob")
                if has_bfc:
                    nc.vector.tensor_tensor(out=ob[:], in0=po[:],
                                            in1=bfc_sb[:], op=ALU.add)
                else:
                    nc.scalar.activation(out=ob[:], in_=po[:], func=AFT.Copy)
                nc.sync.dma_start(out=out_d[w * P:(w + 1) * P, :], in_=ob[:])

        # ---------------- layer schedule with dripped phase-1 --------------
        drip = []

        def drip_emit(n):
            for _ in range(min(n, len(drip))):
                p1_group(*drip.pop(0))

        for l in range(L):
            wq_l = wq_sb[:, l * C:(l + 1) * C]
            if l == 0:
                for args in groups_of(0, 0, kvA_l[0]):
                    p1_group(*args)            # kvA0 eager (first dependency)
                drip = list(groups_of(0, 1, kvB_l[0]))
            # local q for this layer (reads hloc; cheap)
            for w in range(W):
                pq = ps.tile([P, KV], f32, tag="pk")
                nc.tensor.matmul(pq[:, :C], lhsT=hloc[:, w * P:(w + 1) * P],
                                 rhs=wq_l, start=True, stop=True)
                if has_bq:
                    nc.vector.tensor_tensor(
                        out=qloc[:, w * C:(w + 1) * C], in0=pq[:, :C],
                        in1=bq_sb[:, l * C:(l + 1) * C], op=ALU.add)
                else:
                    nc.scalar.activation(out=qloc[:, w * C:(w + 1) * C],
                                         in_=pq[:, :C], func=AFT.Copy)
            # pass 1 (low halves), dripping deferred phase-1 builds
            prev = None
            for w in range(W):
                cur = stage_a(l, w, True)
                stage_b(prev)
                prev = cur
                if l == 0 or w >= 34:
                    drip_emit(5)
            stage_b(prev)
            drip_emit(10 ** 9)
            # pass 2 (high halves) + epilogues; during l==0 drip layer-1 kvA
            if l == 0:
                drip = list(groups_of(1, 0, kvA_l[1]))
            prev = None
            for w in range(W):
                cur = stage_a(l, w, False)
                stage_b(prev)
                prev = cur
                if l == 0 and w >= W2 + 30:
                    drip_emit(6)
            stage_b(prev)
            if l == 0:
                drip_emit(10 ** 9)
                drip = list(groups_of(1, 1, kvB_l[1]))

    nc.compile()
    return nc


def _prep_host(x, edge_index, Wk, bk, Wq, bq, Wv, bv, a_rel, m_rel, p_rel,
               Wa, ba, skip, Wfc, bfc):
    import ml_dtypes
    bfd = ml_dtypes.bfloat16
    N = x.shape[0]
    SH = int(math.ceil(N / NCORES / P / 2)) * P * 2
    W = SH // P
    W2 = W // 2
    SH2 = SH // 2
    NPAD = NCORES * SH

    Wk_eff = np.einsum("lchd,lhde->lche", Wk.reshape(L, C, H, D),
                       a_rel, optimize=True).reshape(L, C, C)
    bk_eff = np.einsum("lhd,lhde->lhe", bk.reshape(L, H, D), a_rel).reshape(L, C)
    Wv_eff = np.einsum("lchd,lhde->lche", Wv.reshape(L, C, H, D),
                       m_rel, optimize=True).reshape(L, C, C)
    bv_eff = np.einsum("lhd,lhde->lhe", bv.reshape(L, H, D), m_rel).reshape(L, C)
    scale = (p_rel / np.sqrt(D)).astype(np.float32)
    Wq_eff = (Wq.reshape(L, C, H, D) * scale[:, None, :, None]).reshape(L, C, C)
    bq_eff = (bq.reshape(L, H, D) * scale[:, :, None]).reshape(L, C)
    g_vals = [float(1.0 / (1.0 + np.exp(-skip[l]))) for l in range(L)]

    dh = (np.arange(C) % H) * D + np.arange(C) // H
    Wv2 = Wv_eff[:, :, dh]
    bv2 = bv_eff[:, dh]
    Wkv = np.concatenate([Wk_eff, Wv2], axis=2)
    bkv = np.concatenate([bk_eff, bv2], axis=1)
    Wa_eff = np.stack([g_vals[l] * Wa[l][dh, :] for l in range(L)])
    bag = np.stack([g_vals[l] * ba[l] for l in range(L)])

    src = np.asarray(edge_index[0], np.int64)
    dst = np.asarray(edge_index[1], np.int64)
    core = dst // SH

    percore = []
    Tlo_all = np.zeros([NCORES, W], np.int64)
    Thi_all = np.zeros([NCORES, W], np.int64)
    for m in range(NCORES):
        sel = core == m
        d = (dst[sel] - m * SH).astype(np.int32)
        s_ = src[sel].astype(np.int32)
        half = ((s_ % SH) >= SH2).astype(np.int32)
        o = np.lexsort((d, half, d >> 7))
        d = d[o]
        s_ = s_[o]
        half = half[o]
        win = d >> 7
        grp = win * 2 + half
        cnt = np.bincount(grp, minlength=2 * W)
        Tlo_all[m] = (cnt[0::2] + P - 1) // P
        Thi_all[m] = (cnt[1::2] + P - 1) // P
        percore.append((d, s_, half, win, grp, cnt))
    Tlo = tuple(int(t) for t in Tlo_all.max(axis=0))
    Thi = tuple(int(t) for t in Thi_all.max(axis=0))
    Tw = np.array(Tlo) + np.array(Thi)
    ofs = np.zeros(W + 1, np.int64)
    ofs[1:] = np.cumsum(Tw)
    CT = int(ofs[-1])

    xTb = np.zeros([P, NPAD], bfd)
    xTb[:, :N] = np.ascontiguousarray(x.T).astype(bfd)

    in_maps = []
    for m in range(NCORES):
        d, s_, half, win, grp, cnt = percore[m]
        srcix = np.zeros([P, CT], np.int32)
        dcol = np.full([P, CT], -1, np.float32)
        lo = half == 0
        ncnt_lo = np.bincount(d[lo], minlength=SH).reshape(W, P)
        ncnt_hi = np.bincount(d[~lo], minlength=SH).reshape(W, P)
        r1a = np.cumsum(ncnt_lo, axis=1).astype(np.float32)
        r0a = r1a - ncnt_lo
        r1b = np.cumsum(ncnt_hi, axis=1).astype(np.float32)
        r0b = r1b - ncnt_hi
        r01 = np.stack([r0a, r1a, r0b, r1b], axis=2).astype(np.float32)
        if len(d):
            starts = np.zeros(2 * W, np.int64)
            starts[1:] = np.cumsum(cnt)[:-1]
            j = np.arange(len(d)) - starts[grp]
            t = (j >> 7).astype(np.int64)
            p = (j & 127).astype(np.int64)
            cbase = np.where(half == 0, ofs[win], ofs[win] + np.array(Tlo)[win])
            col = cbase + t
            shard = s_ // SH
            iloc = s_ % SH
            row = shard * SH2 + np.where(half == 0, iloc, iloc - SH2)
            srcix[p, col] = row
            dcol[p, col] = (d & 127).astype(np.float32)
        im = {
            "xT": xTb,
            "hL0": np.ascontiguousarray(xTb[:, m * SH:(m + 1) * SH]),
            "srcix": srcix,
            "dcol": dcol,
            "r01": np.ascontiguousarray(r01),
            "Wkv": np.ascontiguousarray(Wkv).astype(bfd),
            "Wq": np.ascontiguousarray(Wq_eff).astype(bfd),
            "Wa": np.ascontiguousarray(Wa_eff).astype(bfd),
            "Wfc": np.ascontiguousarray(Wfc).astype(bfd),
        }
        flags = dict(
            has_bkv=bool(np.any(bkv != 0)),
            has_bq=bool(np.any(bq_eff != 0)),
            has_ba=bool(np.any(bag != 0)),
            has_bfc=bool(np.any(bfc != 0)),
        )
        if flags["has_bkv"]:
            im["bkv"] = np.ascontiguousarray(
                np.broadcast_to(bkv[:, None, :], (L, P, KV))).astype(bfd)
        if flags["has_bq"]:
            im["bq"] = np.ascontiguousarray(
                np.broadcast_to(bq_eff[:, None, :], (L, P, C))).astype(bfd)
        if flags["has_ba"]:
            im["bag"] = np.ascontiguousarray(bag[:, dh, None], dtype=np.float32)
        if flags["has_bfc"]:
            im["bfc"] = np.ascontiguousarray(
                np.broadcast_to(bfc[None, :], (P, OUT)), dtype=np.float32)
        in_maps.append(im)

    return SH, W, NPAD, Tlo, Thi, g_vals, in_maps, flags


def kernel(x, edge_index, Wk, bk, Wq, bq, Wv, bv, a_rel, m_rel, p_rel,
           Wa, ba, skip, Wfc, bfc, trace=False):
    global LAST_RESULTS
    x = np.asarray(x, np.float32)
    args = [np.asarray(a, np.float32) for a in
            (Wk, bk, Wq, bq, Wv, bv, a_rel, m_rel, p_rel, Wa, ba, skip, Wfc, bfc)]
    N = x.shape[0]

    SH, W, NPAD, Tlo, Thi, g_vals, in_maps, flags = _prep_host(
        x, edge_index, *args)
    key = (SH, W, NPAD, Tlo, Thi, tuple(g_vals), tuple(sorted(flags.items())))
    nc = _NC_CACHE.get(key)
    if nc is None:
        nc = _build(SH, W, NPAD, Tlo, Thi, g_vals, **flags)
        _NC_CACHE[key] = nc
    try:
        res = run_bass_kernel_spmd(nc, in_maps, list(range(NCORES)), trace=trace)
    except ModuleNotFoundError:
        res = run_bass_kernel_spmd(nc, in_maps, list(range(NCORES)), trace=False)
    LAST_RESULTS = res

    out = np.empty([N, OUT], np.float32)
    for m in range(NCORES):
        lo = m * SH
        hi = min(N, lo + SH)
        if hi > lo:
            out[lo:hi] = res.results[m]["out"][:hi - lo]
    return out


# revision 4
# speedup vs baseline: 1.6539x; 1.1067x over previous
"""HGT on 8 trn2 NeuronCores — bf16, two-pass phase-2 pipeline (v5).

Dst-sharded node partition; kv tables split by src node half (kvA/kvB,
per-layer tensors). Phase 2 runs in two passes over all windows:
  pass 1: low-half columns only; partial (agg|ez-sum) spilled PSUM->SBUF.
  pass 2: high-half columns; partial reinjected via identity matmul; epilogue.
Interleaved ("dripped") phase-1 builds and two chunked h AllGathers keep the
Pool engine's gather-issue stream (the hard bottleneck at ~1m04ns per
128-edge column) running with almost no stalls across layer boundaries.
"""

import math
import sys
from contextlib import ExitStack

sys.path.insert(0, "/opt/trn_rl_repo")

import numpy as np

from concourse import bacc, bass, mybir
from concourse.bass_utils import run_bass_kernel_spmd
from concourse.masks import make_identity
from concourse.tile import TileContext

NCORES = 8
P = 128
C = 128
H = 4
D = 32
L = 2
OUT = 2
KV = 2 * C
THCAP = 16

f32 = mybir.dt.float32
bf16 = mybir.dt.bfloat16
i32 = mybir.dt.int32
i16 = mybir.dt.int16

LAST_RESULTS = None
_NC_CACHE = {}


def _ap(base, pattern):
    return bass.AP(base.tensor, base.offset, pattern)


def _build(SH, W, NPAD, Tlo, Thi, g_vals, has_bkv, has_bq, has_ba, has_bfc):
    nc = bacc.Bacc("TRN2", target_bir_lowering=False)
    W2 = W // 2
    SH2 = SH // 2
    NP2 = NPAD // 2
    Tw = [a + b for a, b in zip(Tlo, Thi)]
    TH = max(max(Tlo), max(Thi))
    assert TH <= THCAP
    ofs = [0]
    for t in Tw:
        ofs.append(ofs[-1] + t)
    CT = ofs[-1]
    ofs2 = {}
    acc = 0
    for w in range(W):
        ofs2[(w, True)] = acc
        acc += Tlo[w] + 2
        ofs2[(w, False)] = acc
        acc += Thi[w] + 2

    xT = nc.dram_tensor("xT", [P, NPAD], bf16, kind="ExternalInput")
    hL0 = nc.dram_tensor("hL0", [P, SH], bf16, kind="ExternalInput")
    srcix_d = nc.dram_tensor("srcix", [P, CT], i32, kind="ExternalInput")
    dcol_d = nc.dram_tensor("dcol", [P, CT + 4 * W], f32, kind="ExternalInput")
    Wkv_d = nc.dram_tensor("Wkv", [L, C, KV], bf16, kind="ExternalInput")
    Wq_d = nc.dram_tensor("Wq", [L, C, C], bf16, kind="ExternalInput")
    Wa_d = nc.dram_tensor("Wa", [L, C, C], bf16, kind="ExternalInput")
    Wfc_d = nc.dram_tensor("Wfc", [C, OUT], bf16, kind="ExternalInput")
    if has_bkv:
        bkv_d = nc.dram_tensor("bkv", [L, P, KV], bf16, kind="ExternalInput")
    if has_bq:
        bq_d = nc.dram_tensor("bq", [L, P, C], bf16, kind="ExternalInput")
    if has_ba:
        bag_d = nc.dram_tensor("bag", [L, C, 1], f32, kind="ExternalInput")
    if has_bfc:
        bfc_d = nc.dram_tensor("bfc", [P, OUT], f32, kind="ExternalInput")
    out_d = nc.dram_tensor("out", [SH, OUT], f32, kind="ExternalOutput")

    kvA_l = [nc.dram_tensor(f"kvA{l}", [NP2, KV], bf16) for l in range(L)]
    kvB_l = [nc.dram_tensor(f"kvB{l}", [NP2, KV], bf16) for l in range(L)]
    h1a = nc.dram_tensor("h1a", [P, SH2], bf16)
    h1b = nc.dram_tensor("h1b", [P, SH2], bf16)
    ag_a = nc.dram_tensor("ag_a", [NCORES, P, SH2], bf16, addr_space="Shared")
    ag_b = nc.dram_tensor("ag_b", [NCORES, P, SH2], bf16, addr_space="Shared")

    AFT = mybir.ActivationFunctionType
    ALU = mybir.AluOpType
    TP = TH * P

    with TileContext(nc) as tc, ExitStack() as ctx:
        cpool = ctx.enter_context(tc.tile_pool(name="consts", bufs=1))
        p1 = ctx.enter_context(tc.tile_pool(name="p1", bufs=3))
        pwin = ctx.enter_context(tc.tile_pool(name="pwin", bufs=3))
        pgath = ctx.enter_context(tc.tile_pool(name="pgath", bufs=8))
        pepi = ctx.enter_context(tc.tile_pool(name="pepi", bufs=2))
        ps = ctx.enter_context(tc.tile_pool(name="ps", bufs=2, space="PSUM"))

        ident = cpool.tile([P, P], bf16)
        make_identity(nc, ident[:])
        iota128 = cpool.tile([P, P], i16)
        nc.gpsimd.iota(iota128[:], pattern=[[1, P]], base=0, channel_multiplier=0)
        iotaF = cpool.tile([P, TP], i16)
        nc.gpsimd.iota(iotaF[:], pattern=[[1, TP]], base=0, channel_multiplier=0)
        iota128f = cpool.tile([P, P], f32)
        nc.scalar.activation(out=iota128f[:], in_=iota128[:], func=AFT.Copy)

        wkv_sb = cpool.tile([P, L * KV], bf16)
        wq_sb = cpool.tile([P, L * C], bf16)
        wa_sb = cpool.tile([P, L * C], bf16)
        wfc_sb = cpool.tile([P, OUT], bf16)
        for l in range(L):
            nc.sync.dma_start(out=wkv_sb[:, l * KV:(l + 1) * KV], in_=Wkv_d[l])
            nc.sync.dma_start(out=wq_sb[:, l * C:(l + 1) * C], in_=Wq_d[l])
            nc.sync.dma_start(out=wa_sb[:, l * C:(l + 1) * C], in_=Wa_d[l])
        nc.sync.dma_start(out=wfc_sb[:], in_=Wfc_d[:])
        if has_bkv:
            bkv_sb = cpool.tile([P, L * KV], bf16)
            for l in range(L):
                nc.sync.dma_start(out=bkv_sb[:, l * KV:(l + 1) * KV], in_=bkv_d[l])
        if has_bq:
            bq_sb = cpool.tile([P, L * C], bf16)
            for l in range(L):
                nc.sync.dma_start(out=bq_sb[:, l * C:(l + 1) * C], in_=bq_d[l])
        if has_ba:
            bag_sb = cpool.tile([P, L], f32)
            for l in range(L):
                nc.sync.dma_start(out=bag_sb[:, l:l + 1], in_=bag_d[l])
        if has_bfc:
            bfc_sb = cpool.tile([P, OUT], f32)
            nc.sync.dma_start(out=bfc_sb[:], in_=bfc_d[:])

        hloc = cpool.tile([P, SH], bf16)
        nc.sync.dma_start(out=hloc[:], in_=hL0[:, :])
        qloc = cpool.tile([P, SH], bf16)
        partial = cpool.tile([P, W * 132], bf16)

        def p1_group(l, half, kvX, s, g0):
            wkv_l = wkv_sb[:, l * KV:(l + 1) * KV]
            gl = min(8, W2 - g0)
            if l == 0:
                c0 = s * SH + half * SH2 + g0 * P
                src_ap = xT[:, c0:c0 + gl * P]
            else:
                agx = ag_a if half == 0 else ag_b
                src_ap = agx[s][:, g0 * P:(g0 + gl) * P]
            ht = p1.tile([P, 8 * P], bf16, tag="ht")
            nc.sync.dma_start(out=ht[:, :gl * P], in_=src_ap)
            kvb = p1.tile([P, 8 * KV], bf16, tag="kvb")
            for i in range(gl):
                pk = ps.tile([P, KV], f32, tag="pk")
                nc.tensor.matmul(pk[:], lhsT=ht[:, i * P:(i + 1) * P],
                                 rhs=wkv_l, start=True, stop=True)
                dst = kvb[:, i * KV:(i + 1) * KV]
                if has_bkv:
                    nc.vector.tensor_tensor(
                        out=dst, in0=pk[:],
                        in1=bkv_sb[:, l * KV:(l + 1) * KV], op=ALU.add)
                elif (g0 // 8 + i) % 2 == 0:
                    nc.scalar.activation(out=dst, in_=pk[:], func=AFT.Copy)
                else:
                    nc.vector.tensor_copy(dst, pk[:])
            row = s * SH2 + g0 * P
            base = kvX[row:row + P, :]
            nc.sync.dma_start(
                out=bass.AP(base.tensor, base.offset,
                            [[KV, P], [P * KV, gl], [1, KV]]),
                in_=kvb[:, :gl * KV])

        def groups_of(l, half, kvX):
            return [(l, half, kvX, s, g0)
                    for s in range(NCORES) for g0 in range(0, W2, 8)]

        def stage_a(l, w, lo):
            """Loads, gathers, masks, q-select for one window half."""
            tlo = Tlo[w]
            tn = tlo if lo else Thi[w]
            o = ofs[w] if lo else ofs[w] + tlo
            kvX = kvA_l[l] if lo else kvB_l[l]
            if tn == 0 and lo:
                nc.vector.memset(partial[:, w * 132:(w + 1) * 132], 0)
                return None
            tp = tn * P
            ctx = dict(l=l, w=w, lo=lo, tn=tn)
            if tn:
                six = pgath.tile([P, TH], i32, tag="six")
                nc.sync.dma_start(out=six[:, :tn], in_=srcix_d[:, o:o + tn])
                o2 = ofs2[(w, lo)]
                dct = pgath.tile([P, TH + 2], f32, tag="dct")
                nc.sync.dma_start(out=dct[:, :tn + 2],
                                  in_=dcol_d[:, o2:o2 + tn + 2])
                kva = pgath.tile([P, TH * KV], bf16, tag="kva")
                for t in range(tn):
                    nc.gpsimd.indirect_dma_start(
                        out=kva[:, t * KV:(t + 1) * KV], out_offset=None,
                        in_=kvX[:, :],
                        in_offset=bass.IndirectOffsetOnAxis(
                            ap=six[:, t:t + 1], axis=0))
                ST = pwin.tile([P, TP], bf16, tag="ST")
                nc.vector.tensor_scalar(
                    out=ST[:, :tp], in0=iotaF[:, :tp],
                    scalar1=dct[:, tn + 1:tn + 2], scalar2=None, op0=ALU.is_lt)
                nc.vector.scalar_tensor_tensor(
                    out=ST[:, :tp], in0=iotaF[:, :tp],
                    scalar=dct[:, tn:tn + 1],
                    in1=ST[:, :tp], op0=ALU.is_ge, op1=ALU.mult)
                S = pwin.tile([P, TP], bf16, tag="S")
                nc.vector.tensor_tensor(
                    out=S[:, :tp].rearrange("p (t n) -> p t n", n=P),
                    in0=dct[:, :tn].to_broadcast([P, tn, P]),
                    in1=_ap(iota128f[:], [[P, P], [0, tn], [1, P]]),
                    op=ALU.is_equal)
                qw = qloc[:, w * C:(w + 1) * C]
                qsb = pwin.tile([P, TP], bf16, tag="qsb")
                t0 = 0
                while t0 < tn:
                    gl = min(4, tn - t0)
                    psq = ps.tile([P, 512], f32, tag="psq")
                    for i in range(gl):
                        t = t0 + i
                        nc.tensor.matmul(psq[:, i * P:(i + 1) * P],
                                         lhsT=ST[:, t * P:(t + 1) * P],
                                         rhs=qw, start=True, stop=True)
                    nc.scalar.activation(out=qsb[:, t0 * P:(t0 + gl) * P],
                                         in_=psq[:, :gl * P], func=AFT.Copy)
                    t0 += gl
                ctx.update(kva=kva, S=S, qsb=qsb)
            return ctx

        def stage_b(ctx):
            """Products, softmax pieces, scatter accumulation, epilogue."""
            if ctx is None:
                return
            l, w, lo, tn = ctx["l"], ctx["w"], ctx["lo"], ctx["tn"]
            g = g_vals[l]
            wa_l = wa_sb[:, l * C:(l + 1) * C]
            ags = ps.tile([P, 132], f32, tag="ags")
            if tn:
                kva, S, qsb = ctx["kva"], ctx["S"], ctx["qsb"]
                tp = tn * P
                kva3 = kva[:].rearrange("p (t c) -> p t c", c=KV)
                prod = pwin.tile([P, TP], bf16, tag="prod")
                pv = prod[:].rearrange("p (t h d) -> p t h d", h=H, d=D)
                nc.vector.tensor_tensor(
                    out=pv[:, :tn],
                    in0=qsb[:, :tp].rearrange("p (t h d) -> p t h d", h=H, d=D),
                    in1=kva3[:, :tn, 0:C].rearrange("p t (h d) -> p t h d", d=D),
                    op=ALU.mult)
                f1 = pwin.tile([P, TH * H * 16], bf16, tag="f1")
                f1v = f1[:].rearrange("p (t h d) -> p t h d", h=H, d=16)
                nc.vector.tensor_tensor(out=f1v[:, :tn], in0=pv[:, :tn, :, 0:16],
                                        in1=pv[:, :tn, :, 16:32], op=ALU.add)
                f2 = pwin.tile([P, TH * H * 8], bf16, tag="f2")
                f2v = f2[:].rearrange("p (t h d) -> p t h d", h=H, d=8)
                nc.vector.tensor_tensor(out=f2v[:, :tn], in0=f1v[:, :tn, :, 0:8],
                                        in1=f1v[:, :tn, :, 8:16], op=ALU.add)
                f3 = pwin.tile([P, TH * H * 4], bf16, tag="f3")
                f3v = f3[:].rearrange("p (t h d) -> p t h d", h=H, d=4)
                nc.vector.tensor_tensor(out=f3v[:, :tn], in0=f2v[:, :tn, :, 0:4],
                                        in1=f2v[:, :tn, :, 4:8], op=ALU.add)
                f4 = pwin.tile([P, TH * H * 2], bf16, tag="f4")
                f4v = f4[:].rearrange("p (t h d) -> p t h d", h=H, d=2)
                nc.vector.tensor_tensor(out=f4v[:, :tn], in0=f3v[:, :tn, :, 0:2],
                                        in1=f3v[:, :tn, :, 2:4], op=ALU.add)
                alpha = pwin.tile([P, TH * H], bf16, tag="alpha")
                av = alpha[:].rearrange("p (t h) -> p t h", h=H)
                nc.vector.tensor_tensor(out=av[:, :tn],
                                        in0=f4v[:, :tn, :, 0:1],
                                        in1=f4v[:, :tn, :, 1:2], op=ALU.add)
                msg = pwin.tile([P, TH * 132], bf16, tag="msg")
                mv = msg[:].rearrange("p (t c) -> p t c", c=132)
                nc.scalar.activation(out=mv[:, :tn, C:C + 4],
                                     in_=av[:, :tn], func=AFT.Exp)
                ez_ap = _ap(msg[:, C:C + 4],
                            [[TH * 132, P], [132, tn], [0, D], [1, H]])
                nc.vector.tensor_tensor(
                    out=mv[:, :tn, 0:C].rearrange("p t (d h) -> p t d h", h=H),
                    in0=kva3[:, :tn, C:KV].rearrange("p t (d h) -> p t d h", h=H),
                    in1=ez_ap,
                    op=ALU.mult)
            if lo:
                if tn:
                    for t in range(tn):
                        nc.tensor.matmul(ags[:, 0:132],
                                         lhsT=S[:, t * P:(t + 1) * P],
                                         rhs=msg[:, t * 132:(t + 1) * 132],
                                         start=(t == 0), stop=(t == tn - 1),
                                         skip_group_check=True)
                    nc.vector.tensor_copy(partial[:, w * 132:(w + 1) * 132],
                                          ags[:, 0:132])
                return
            nc.tensor.matmul(ags[:, 0:132], lhsT=ident[:],
                             rhs=partial[:, w * 132:(w + 1) * 132],
                             start=True, stop=(tn == 0), skip_group_check=True)
            for t in range(tn):
                nc.tensor.matmul(ags[:, 0:132],
                                 lhsT=S[:, t * P:(t + 1) * P],
                                 rhs=msg[:, t * 132:(t + 1) * 132],
                                 start=False, stop=(t == tn - 1),
                                 skip_group_check=True)

            den = pepi.tile([P, 4], f32, tag="den")
            nc.vector.tensor_scalar_max(den[:], ags[:, C:C + 4], 1e-30)
            rec = pepi.tile([P, 4], f32, tag="rec")
            nc.vector.reciprocal(rec[:], den[:])
            aggn = pepi.tile([P, C], bf16, tag="aggn")
            nc.vector.tensor_tensor(
                out=aggn[:].rearrange("p (d h) -> p d h", h=H),
                in0=ags[:, 0:C].rearrange("p (d h) -> p d h", h=H),
                in1=_ap(rec[:], [[4, P], [0, D], [1, H]]),
                op=ALU.mult)
            gact = pepi.tile([P, C], bf16, tag="gact")
            nc.scalar.activation(out=gact[:], in_=aggn[:], func=AFT.Gelu)
            gt = ps.tile([P, 2 * P], bf16, tag="epi")
            nc.tensor.transpose(gt[:, :P], gact[:], ident[:])
            gts = pepi.tile([P, P], bf16, tag="gts")
            nc.scalar.activation(out=gts[:], in_=gt[:, :P], func=AFT.Copy)
            op_ = ps.tile([P, P], f32, tag="epi")
            nc.tensor.matmul(op_[:], lhsT=wa_l, rhs=gts[:],
                             start=True, stop=True)
            hsl = hloc[:, w * P:(w + 1) * P]
            nc.vector.scalar_tensor_tensor(
                out=hsl, in0=hsl, scalar=float(1.0 - g), in1=op_[:],
                op0=ALU.mult, op1=ALU.add)
            if has_ba:
                nc.vector.tensor_tensor(
                    out=hsl, in0=hsl,
                    in1=bag_sb[:, l:l + 1].to_broadcast([P, P]), op=ALU.add)
            if l == 0:
                if w < W2:
                    nc.sync.dma_start(out=h1a[:, w * P:(w + 1) * P], in_=hsl)
                else:
                    nc.sync.dma_start(
                        out=h1b[:, (w - W2) * P:(w - W2 + 1) * P], in_=hsl)
                if w == min(W2 + 5, W - 1):
                    nc.gpsimd.collective_compute(
                        "AllGather", ALU.bypass,
                        replica_groups=[list(range(NCORES))],
                        ins=[h1a[:]], outs=[ag_a[:]])
                if w == W - 1:
                    nc.gpsimd.collective_compute(
                        "AllGather", ALU.bypass,
                        replica_groups=[list(range(NCORES))],
                        ins=[h1b[:]], outs=[ag_b[:]])
            else:
                po = ps.tile([P, OUT], f32, tag="epi")
                nc.tensor.matmul(po[:], lhsT=hsl, rhs=wfc_sb[:],
                                 start=True, stop=True)
                ob = pepi.tile([P, OUT], f32, tag="ob")
                if has_bfc:
                    nc.vector.tensor_tensor(out=ob[:], in0=po[:],
                                            in1=bfc_sb[:], op=ALU.add)
                else:
                    nc.scalar.activation(out=ob[:], in_=po[:], func=AFT.Copy)
                nc.sync.dma_start(out=out_d[w * P:(w + 1) * P, :], in_=ob[:])

        # ---------------- layer schedule with dripped phase-1 --------------
        drip = []

        def drip_emit(n):
            for _ in range(min(n, len(drip))):
                p1_group(*drip.pop(0))

        for l in range(L):
            wq_l = wq_sb[:, l * C:(l + 1) * C]
            if l == 0:
                for args in groups_of(0, 0, kvA_l[0]):
                    p1_group(*args)            # kvA0 eager (first dependency)
                drip = list(groups_of(0, 1, kvB_l[0]))
            # local q for this layer (reads hloc; cheap)
            for w in range(W):
                pq = ps.tile([P, KV], f32, tag="pk")
                nc.tensor.matmul(pq[:, :C], lhsT=hloc[:, w * P:(w + 1) * P],
                                 rhs=wq_l, start=True, stop=True)
                if has_bq:
                    nc.vector.tensor_tensor(
                        out=qloc[:, w * C:(w + 1) * C], in0=pq[:, :C],
                        in1=bq_sb[:, l * C:(l + 1) * C], op=ALU.add)
                else:
                    nc.scalar.activation(out=qloc[:, w * C:(w + 1) * C],
                                         in_=pq[:, :C], func=AFT.Copy)
            # pass 1 (low halves), dripping deferred phase-1 builds
            prev = None
            for w in range(W):
                cur = stage_a(l, w, True)
                stage_b(prev)
                prev = cur
                if l == 0 or w >= 34:
                    drip_emit(3)
            stage_b(prev)
            drip_emit(10 ** 9)
            # pass 2 (high halves) + epilogues; during l==0 drip layer-1 kvA
            if l == 0:
                drip = list(groups_of(1, 0, kvA_l[1]))
            prev = None
            for w in range(W):
                cur = stage_a(l, w, False)
                stage_b(prev)
                prev = cur
                if l == 0 and w >= W2 + 30:
                    drip_emit(3)
            stage_b(prev)
            if l == 0:
                drip_emit(10 ** 9)
                drip = list(groups_of(1, 1, kvB_l[1]))

    nc.compile()
    return nc


def _prep_host(x, edge_index, Wk, bk, Wq, bq, Wv, bv, a_rel, m_rel, p_rel,
               Wa, ba, skip, Wfc, bfc):
    import ml_dtypes
    bfd = ml_dtypes.bfloat16
    N = x.shape[0]
    SH = int(math.ceil(N / NCORES / P / 2)) * P * 2
    W = SH // P
    W2 = W // 2
    SH2 = SH // 2
    NPAD = NCORES * SH

    Wk_eff = np.einsum("lchd,lhde->lche", Wk.reshape(L, C, H, D),
                       a_rel, optimize=True).reshape(L, C, C)
    bk_eff = np.einsum("lhd,lhde->lhe", bk.reshape(L, H, D), a_rel).reshape(L, C)
    Wv_eff = np.einsum("lchd,lhde->lche", Wv.reshape(L, C, H, D),
                       m_rel, optimize=True).reshape(L, C, C)
    bv_eff = np.einsum("lhd,lhde->lhe", bv.reshape(L, H, D), m_rel).reshape(L, C)
    scale = (p_rel / np.sqrt(D)).astype(np.float32)
    Wq_eff = (Wq.reshape(L, C, H, D) * scale[:, None, :, None]).reshape(L, C, C)
    bq_eff = (bq.reshape(L, H, D) * scale[:, :, None]).reshape(L, C)
    g_vals = [float(1.0 / (1.0 + np.exp(-skip[l]))) for l in range(L)]

    dh = (np.arange(C) % H) * D + np.arange(C) // H
    Wv2 = Wv_eff[:, :, dh]
    bv2 = bv_eff[:, dh]
    Wkv = np.concatenate([Wk_eff, Wv2], axis=2)
    bkv = np.concatenate([bk_eff, bv2], axis=1)
    Wa_eff = np.stack([g_vals[l] * Wa[l][dh, :] for l in range(L)])
    bag = np.stack([g_vals[l] * ba[l] for l in range(L)])

    src = np.asarray(edge_index[0], np.int64)
    dst = np.asarray(edge_index[1], np.int64)
    core = dst // SH

    percore = []
    Tlo_all = np.zeros([NCORES, W], np.int64)
    Thi_all = np.zeros([NCORES, W], np.int64)
    for m in range(NCORES):
        sel = core == m
        d = (dst[sel] - m * SH).astype(np.int32)
        s_ = src[sel].astype(np.int32)
        half = ((s_ % SH) >= SH2).astype(np.int32)
        o = np.lexsort((d, half, d >> 7))
        d = d[o]
        s_ = s_[o]
        half = half[o]
        win = d >> 7
        grp = win * 2 + half
        cnt = np.bincount(grp, minlength=2 * W)
        Tlo_all[m] = (cnt[0::2] + P - 1) // P
        Thi_all[m] = (cnt[1::2] + P - 1) // P
        percore.append((d, s_, half, win, grp, cnt))
    Tlo = tuple(int(t) for t in Tlo_all.max(axis=0))
    Thi = tuple(int(t) for t in Thi_all.max(axis=0))
    Tw = np.array(Tlo) + np.array(Thi)
    ofs = np.zeros(W + 1, np.int64)
    ofs[1:] = np.cumsum(Tw)
    CT = int(ofs[-1])

    xTb = np.zeros([P, NPAD], bfd)
    xTb[:, :N] = np.ascontiguousarray(x.T).astype(bfd)

    in_maps = []
    for m in range(NCORES):
        d, s_, half, win, grp, cnt = percore[m]
        srcix = np.zeros([P, CT], np.int32)
        dcol = np.full([P, CT + 4 * W], -1, np.float32)
        ofs2lo = np.zeros(W, np.int64)
        ofs2hi = np.zeros(W, np.int64)
        acc = 0
        for w in range(W):
            ofs2lo[w] = acc
            acc += Tlo[w] + 2
            ofs2hi[w] = acc
            acc += Thi[w] + 2
        lo = half == 0
        ncnt_lo = np.bincount(d[lo], minlength=SH).reshape(W, P)
        ncnt_hi = np.bincount(d[~lo], minlength=SH).reshape(W, P)
        r1a = np.cumsum(ncnt_lo, axis=1).astype(np.float32)
        r0a = r1a - ncnt_lo
        r1b = np.cumsum(ncnt_hi, axis=1).astype(np.float32)
        r0b = r1b - ncnt_hi
        for w in range(W):
            dcol[:, ofs2lo[w] + Tlo[w]] = r0a[w]
            dcol[:, ofs2lo[w] + Tlo[w] + 1] = r1a[w]
            dcol[:, ofs2hi[w] + Thi[w]] = r0b[w]
            dcol[:, ofs2hi[w] + Thi[w] + 1] = r1b[w]
        if len(d):
            starts = np.zeros(2 * W, np.int64)
            starts[1:] = np.cumsum(cnt)[:-1]
            j = np.arange(len(d)) - starts[grp]
            t = (j >> 7).astype(np.int64)
            p = (j & 127).astype(np.int64)
            cbase = np.where(half == 0, ofs[win], ofs[win] + np.array(Tlo)[win])
            col = cbase + t
            col2 = np.where(half == 0, ofs2lo[win], ofs2hi[win]) + t
            shard = s_ // SH
            iloc = s_ % SH
            row = shard * SH2 + np.where(half == 0, iloc, iloc - SH2)
            srcix[p, col] = row
            dcol[p, col2] = (d & 127).astype(np.float32)
        im = {
            "xT": xTb,
            "hL0": np.ascontiguousarray(xTb[:, m * SH:(m + 1) * SH]),
            "srcix": srcix,
            "dcol": dcol,
            "Wkv": np.ascontiguousarray(Wkv).astype(bfd),
            "Wq": np.ascontiguousarray(Wq_eff).astype(bfd),
            "Wa": np.ascontiguousarray(Wa_eff).astype(bfd),
            "Wfc": np.ascontiguousarray(Wfc).astype(bfd),
        }
        flags = dict(
            has_bkv=bool(np.any(bkv != 0)),
            has_bq=bool(np.any(bq_eff != 0)),
            has_ba=bool(np.any(bag != 0)),
            has_bfc=bool(np.any(bfc != 0)),
        )
        if flags["has_bkv"]:
            im["bkv"] = np.ascontiguousarray(
                np.broadcast_to(bkv[:, None, :], (L, P, KV))).astype(bfd)
        if flags["has_bq"]:
            im["bq"] = np.ascontiguousarray(
                np.broadcast_to(bq_eff[:, None, :], (L, P, C))).astype(bfd)
        if flags["has_ba"]:
            im["bag"] = np.ascontiguousarray(bag[:, dh, None], dtype=np.float32)
        if flags["has_bfc"]:
            im["bfc"] = np.ascontiguousarray(
                np.broadcast_to(bfc[None, :], (P, OUT)), dtype=np.float32)
        in_maps.append(im)

    return SH, W, NPAD, Tlo, Thi, g_vals, in_maps, flags


def kernel(x, edge_index, Wk, bk, Wq, bq, Wv, bv, a_rel, m_rel, p_rel,
           Wa, ba, skip, Wfc, bfc, trace=False):
    global LAST_RESULTS
    x = np.asarray(x, np.float32)
    args = [np.asarray(a, np.float32) for a in
            (Wk, bk, Wq, bq, Wv, bv, a_rel, m_rel, p_rel, Wa, ba, skip, Wfc, bfc)]
    N = x.shape[0]

    SH, W, NPAD, Tlo, Thi, g_vals, in_maps, flags = _prep_host(
        x, edge_index, *args)
    key = (SH, W, NPAD, Tlo, Thi, tuple(g_vals), tuple(sorted(flags.items())))
    nc = _NC_CACHE.get(key)
    if nc is None:
        nc = _build(SH, W, NPAD, Tlo, Thi, g_vals, **flags)
        _NC_CACHE[key] = nc
    try:
        res = run_bass_kernel_spmd(nc, in_maps, list(range(NCORES)), trace=trace)
    except ModuleNotFoundError:
        res = run_bass_kernel_spmd(nc, in_maps, list(range(NCORES)), trace=False)
    LAST_RESULTS = res

    out = np.empty([N, OUT], np.float32)
    for m in range(NCORES):
        lo = m * SH
        hi = min(N, lo + SH)
        if hi > lo:
            out[lo:hi] = res.results[m]["out"][:hi - lo]
    return out
